# revision 52
# baseline (speedup 1.0000x reference)
"""ContraNorm kernel for 8 Trainium2 NeuronCores — fp8 DoubleRow pipeline.

Math (reference):
    norm_x = x / max(||x||_row, eps)
    sim    = (norm_x @ norm_x.T) / tau          # [N, N], tau = 1
    sim[edge_index[0], edge_index[1]] = -inf
    attn   = softmax(sim, axis=1)
    out    = 1.1 * x - 0.1 * (attn @ x)

Sharding: row-parallel.  Core k owns output rows [k*1024, (k+1)*1024).
Each core receives inputs row-rolled so its own rows sit at c-positions
0:1024 — the program is identical on every core (pure SPMD).

Since sim is a cosine similarity in [-1, 1], softmax needs no running
max.  fp8 (e4m3) everywhere on the matmul paths, DoubleRow perf mode:
  sim:  psum[c,m] = sum_{kt,dp} xt[dp,kt,c] * xt[dp,kt,m]   2 MM / pair
  V:    pv[m,:]  += sum_{kt,cp} et[cp,kt,m] * xa[cp,kt,:]   4 MM / pair
norm_x is pre-scaled by 16 on the host, so psum = 256*sim.  PE work
(1748ns per 2-pair group, ~56us/core warm) is the roofline; the whole
kernel runs as ONE flat 32-group software pipeline (both m-halves),
with exp+mask split so no other engine exceeds the PE:

  pair 0 of each group: TWO DVE scalar_tensor_tensor ops (one per
    single-bank psum tile) each do exp AND mask in one pass:
    u8 = psum*(8/ln2/256) + bias, bias fp8 {56 keep, -240 edge}.
    The u8 bits ARE fp8e4m3(exp(sim)) (Schraudolph: linear-in-mantissa
    exp2, error below the e4m3 quantization step after softmax);
    masked lanes go negative and saturate to 0x00 = +0.0.
  pair 1: TWO ACT exp ops (scale 1/256) -> fp8, then one u32
    bitwise-AND on DVE with a host-shipped {00,FF} keep-mask
    (4 bytes/cycle/lane; the old u8 multiply was 1B/cyc and measured
    70us of DVE — the original bottleneck).

Pipeline schedule (all measured-to-matter):
  - sim psum tiles are SINGLE-BANK [128,512], one per exp sub-op, so
    every psum write->consume->free round trip (~1.1us) sits far below
    the group period — the pipeline cannot fall into the slow
    psum-WAR limit cycle (which otherwise locks at 1964ns/group with
    one LDWEIGHTS-serialized 430ns matmul per group).
  - the AND runs two groups behind its ACT producer and after the
    current group's STTs in the DVE queue; V matmuls run three groups
    behind.  Nothing ever waits at an engine-queue head.
  - 8 warmup matmuls into the (later start=True-cleared) pv bank open
    the HAM clock gate (1.2 -> 2.4 GHz) before the main loop.
  - epilogue: ones column is -10 so DVE reciprocal yields -0.1/S;
    scale-apply alternates ACT Copy (per-partition scale AP) / DVE;
    the +1.1x combine runs on the otherwise-idle GpSimd (xo ships
    pre-scaled); outputs flush in four 256-row DMAs.

Per-core inputs (~13.5 MiB, ~38us DMA, overlapped with compute):
  xt    [128, 2, 8192] fp8     16*norm_x rolled, transposed
  xa    [128, 32, 2, 257] fp8  x rolled (V rhs layout) + (-10)-column
  mk    [2, 128, 4, 4, 2, 1024] u8  per half/partition/slab/group:
        [0] = fp8 exp-bias for pair 0, [1] = {00,FF} AND-mask for
        pair 1
  xo    [1024, 256] f32        1.1 * own rows for the epilogue
"""

import numpy as np
import ml_dtypes

N = 8192          # rows of x
D = 256           # features
P = 128           # SBUF partitions
NT = N // P       # 64 c-chunks
R = N // 8        # 1024 rows per core
HALF = 512        # m columns per pass
NPAIR = NT // 2   # 32 c-chunk pairs
SCALE = 0.1
NCORES = 8
GRP = 2           # pairs per group (1 ACT + 1 DVE)
NGRP = NPAIR // GRP
NSLAB = 4         # mask DMA slabs per half
GPS = NGRP // NSLAB
NWARM = 8         # PE warmup matmuls

SCH_A = 8.0 / np.log(2.0) / 256.0   # psum(=256*sim) -> fp8e4m3 bits slope
B_KEEP = 0x66                       # fp8 bits of +56  (= 8 * e4m3 bias 7)
B_EDGE = 0xF7                       # fp8 bits of -240 (saturates u8 to 0)

_prog_cache = {}


def _build_program():
    import concourse.bacc as bacc
    import concourse.tile as tile
    from concourse import mybir
    from contextlib import ExitStack

    f32 = mybir.dt.float32
    fp8 = mybir.dt.float8e4
    u32 = mybir.dt.uint32
    u8 = mybir.dt.uint8
    DR = mybir.MatmulPerfMode.DoubleRow
    Exp = mybir.ActivationFunctionType.Exp
    Copy = mybir.ActivationFunctionType.Copy
    AND = mybir.AluOpType.bitwise_and
    MUL = mybir.AluOpType.mult
    ADD = mybir.AluOpType.add

    nc = bacc.Bacc("TRN2", target_bir_lowering=False, debug=False)

    xt_h = nc.dram_tensor("xt", [P, 2, N], fp8, kind="ExternalInput")
    xa_h = nc.dram_tensor("xa", [P, NPAIR, 2, D + 1], fp8, kind="ExternalInput")
    mk_h = nc.dram_tensor(
        "mk", [2, P, NSLAB, GPS, 2, 2 * HALF], u8, kind="ExternalInput"
    )
    xo_h = nc.dram_tensor("xo", [R, D], f32, kind="ExternalInput")
    out_h = nc.dram_tensor("out", [R, D], f32, kind="ExternalOutput")

    xo_d = xo_h.ap().rearrange("(j p) d -> p j d", p=P)    # [128, 8, 256]
    out_d = out_h.ap()

    with ExitStack() as ctx:
        tc = ctx.enter_context(tile.TileContext(nc))

        consts = ctx.enter_context(tc.tile_pool(name="consts", bufs=1))
        work = ctx.enter_context(tc.tile_pool(name="work", bufs=4))
        # single-bank sim psum tiles: each STT/ACT sub-op frees its bank
        # ~1.1us after the sim matmul wrote it — every psum round-trip sits
        # far below the 1.75us group period, so the pipeline cannot fall
        # into the slow (psum-WAR -> cold-matmul) limit cycle
        ps_a = ctx.enter_context(tc.tile_pool(name="ps_a", bufs=2, space="PSUM"))
        ps_b = ctx.enter_context(tc.tile_pool(name="ps_b", bufs=2, space="PSUM"))
        ps_v = ctx.enter_context(tc.tile_pool(name="ps_v", bufs=1, space="PSUM"))

        xt = consts.tile([P, 2, N], fp8)
        xa = consts.tile([P, NPAIR, 2, D + 1], fp8)
        xo = consts.tile([P, R // P, D], f32)
        mk = [
            [
                consts.tile([P, GPS, 2, 2 * HALF], u8, name=f"mk{h}{q}")
                for q in range(NSLAB)
            ]
            for h in range(2)
        ]

        # DMA order paces arrival against consumption (~38us total).
        def dx(c0, c1):
            nc.sync.dma_start(out=xt[:, :, c0:c1], in_=xt_h.ap()[:, :, c0:c1])

        XAC = NPAIR // 4
        dx(0, 8 * P)
        # first mask slab ships in two halves so group 0's Schraudolph STT
        # fires ~1.5us earlier (the ramp head is DMA-latency-bound)
        nc.sync.dma_start(out=mk[0][0][:, 0:2], in_=mk_h.ap()[0, :, 0, 0:2])
        dx(8 * P, 16 * P)
        nc.sync.dma_start(out=mk[0][0][:, 2:4], in_=mk_h.ap()[0, :, 0, 2:4])
        nc.sync.dma_start(out=xa[:, 0:XAC], in_=xa_h.ap()[:, 0:XAC])
        nc.sync.dma_start(out=mk[0][1], in_=mk_h.ap()[0, :, 1])
        dx(16 * P, 32 * P)
        nc.sync.dma_start(out=xa[:, XAC:2 * XAC], in_=xa_h.ap()[:, XAC:2 * XAC])
        nc.sync.dma_start(out=mk[0][2], in_=mk_h.ap()[0, :, 2])
        dx(32 * P, 48 * P)
        nc.sync.dma_start(out=xa[:, 2 * XAC:3 * XAC], in_=xa_h.ap()[:, 2 * XAC:3 * XAC])
        nc.sync.dma_start(out=mk[0][3], in_=mk_h.ap()[0, :, 3])
        dx(48 * P, N)
        # the first h1 mask slab's deadline (group 16, ~41us) is ~6us
        # tighter than xa3/xo's — ship it ahead of them (measured +5us
        # stall at the half boundary otherwise)
        nc.sync.dma_start(out=mk[1][0], in_=mk_h.ap()[1, :, 0])
        nc.sync.dma_start(out=xa[:, 3 * XAC:NPAIR], in_=xa_h.ap()[:, 3 * XAC:NPAIR])
        # xo's h1 half isn't read until the h1 epilogue (~75us): defer it
        # past the mask slabs to decongest the 38-48us DMA window
        nc.sync.dma_start(out=xo[:, 0:4], in_=xo_d[:, 0:4])
        for q in range(1, NSLAB):
            nc.sync.dma_start(out=mk[1][q], in_=mk_h.ap()[1, :, q])
        nc.sync.dma_start(out=xo[:, 4:8], in_=xo_d[:, 4:8])

        pvh = {}

        def emit_and(pg, et4m_p, et2_p):
            # mask for the ACT pair, pipelined two groups back and emitted
            # after the current group's STTs — its et2 input is long done,
            # so it never couples the ACT exp latency into the DVE queue
            # ahead of a Schraudolph STT
            if et2_p is None:
                return
            mks = mk[pg // NGRP][(pg % NGRP) // GPS]
            gl = pg % GPS
            nc.vector.tensor_tensor(
                out=et4m_p[:, 1].rearrange("p a b -> p (a b)").bitcast(u32),
                in0=et2_p.rearrange("p a b -> p (a b)").bitcast(u32),
                in1=mks[:, gl, 1, :].bitcast(u32),
                op=AND,
            )

        def emit_v(pg, et, final=False):
            # V matmuls for group pg (software-pipelined three groups back
            # so the PE never queues behind the exp->mask chain).
            # mi-major on the last group: pv[0] stops earlier, so the
            # epilogue chain starts while the tail matmuls run.
            pv = pvh[pg // NGRP]
            gg = pg % NGRP
            order = (
                [(gi, mi) for mi in range(4) for gi in range(GRP)]
                if final
                else [(gi, mi) for gi in range(GRP) for mi in range(4)]
            )
            for gi, mi in order:
                g = gg * GRP + gi
                nc.tensor.matmul(
                    pv[mi],
                    et[:, gi, :, mi * P : (mi + 1) * P],
                    xa[:, g],
                    start=(g == 0),
                    stop=(g == NPAIR - 1),
                    perf_mode=DR,
                )

        def emit_epilogue(h):
            # out = 1.1*x - 0.1 * pv/S; ones column is -10, so
            # pv[:,D] = -10*S and reciprocal gives -0.1/S directly
            pv = pvh[h]
            resh = work.tile([P, 4, D], f32, tag="resh", bufs=2)
            for mi in range(4):
                jj = h * 4 + mi
                sinv = work.tile([P, 1], f32, tag="sinv")
                nc.vector.reciprocal(sinv, pv[mi][:, D : D + 1])
                # split the pv evacuation 2/2 between ACT and DVE (gpsimd
                # cannot read PSUM): a 4-Copy burst on ACT alone delays the
                # next half's exps behind it in the FIFO
                if mi % 2 == 0:
                    nc.scalar.activation(
                        resh[:, mi],
                        pv[mi][:, 0:D],
                        Copy,
                        scale=sinv,
                    )
                else:
                    nc.vector.tensor_scalar(
                        out=resh[:, mi],
                        in0=pv[mi][:, 0:D],
                        scalar1=sinv,
                        scalar2=1.0,
                        op0=MUL,
                        op1=MUL,
                    )
                # float combine on the idle GpSimd engine: keeps the DVE
                # budget (2 STT + AND ~ 1742ns/period) from overflowing at
                # the epilogue.  xo ships pre-scaled by 1.1 so this is a
                # plain add (gpsimd has no scalar_tensor_tensor opcode).
                nc.gpsimd.tensor_tensor(
                    out=resh[:, mi],
                    in0=xo[:, jj],
                    in1=resh[:, mi],
                    op=ADD,
                )
                if mi % 2 == 1:
                    # flush in 256-row chunks so the last output DMA is
                    # small (tail) and the first overlaps the epilogue
                    j0 = h * 4 + mi - 1
                    nc.sync.dma_start(
                        out=out_d[j0 * P : (j0 + 2) * P, :].rearrange(
                            "(j p) d -> p j d", p=P
                        ),
                        in_=resh[:, mi - 1 : mi + 1],
                    )

        # single flat 32-group software pipeline across both halves — no
        # drain/refill dip at the half boundary
        groups = []
        NU = 2 * NGRP
        for u in range(NU):
            h, gg = divmod(u, NGRP)
            m0 = h * HALF
            if gg == 0:
                pvh[h] = [
                    ps_v.tile([P, D + 1], f32, tag=f"pv{mi}", name=f"pv{h}{mi}")
                    for mi in range(4)
                ]
            if u == 0:
                # PE warmup: junk matmuls into pv0 (cleared by the real
                # start=True) so the HAM clock gate opens before the
                # main loop needs full throughput.
                for _ in range(NWARM):
                    nc.tensor.matmul(
                        pvh[0][0][:, 0:D],
                        xt[:, 0, 0:P],
                        xt[:, 0, 0:2 * P],
                        start=True,
                        stop=True,
                    )
            et4m = work.tile([P, GRP, 2, HALF], fp8, tag="et4m", bufs=5)
            mks = mk[h][gg // GPS]
            gl = gg % GPS
            et2 = None
            for gi in range(GRP):
                g = gg * GRP + gi
                stt_pair = gi == 0
                if not stt_pair:
                    et2 = work.tile([P, 2, HALF], fp8, tag="et2", bufs=5)
                for kt in range(2):
                    t = 2 * g + kt
                    pool = ps_a if gi == 0 else ps_b
                    pss = pool.tile([P, HALF], f32, tag=f"ps{gi}")
                    nc.tensor.matmul(
                        pss,
                        xt[:, :, t * P : (t + 1) * P],
                        xt[:, :, m0 : m0 + HALF],
                        start=True,
                        stop=True,
                        perf_mode=DR,
                    )
                    if stt_pair:
                        # one DVE op = Schraudolph exp + mask (first pair
                        # always; both pairs in the very last group so the
                        # tail never waits on ACT exp + AND)
                        nc.vector.scalar_tensor_tensor(
                            out=et4m[:, gi, kt, :].bitcast(u8),
                            in0=pss,
                            scalar=float(SCH_A),
                            in1=mks[:, gl, gi, kt * HALF : (kt + 1) * HALF].bitcast(fp8),
                            op0=MUL,
                            op1=ADD,
                        )
                    else:
                        # second pair: ACT exp -> fp8, masked by emit_and
                        nc.scalar.activation(
                            et2[:, kt, :],
                            pss,
                            Exp,
                            scale=1.0 / 256.0,
                        )
            groups.append((u, et4m, et2))
            if u >= 2:
                emit_and(*groups[u - 2])
            if u >= 3:
                emit_v(u - 3, groups[u - 3][1])
                if (u - 3) % NGRP == NGRP - 1:
                    emit_epilogue((u - 3) // NGRP)
        # tail: ANDs as early as their ACT inputs allow, so the trailing
        # V groups and the epilogue recips behind them on DVE stream
        # without queue-head waits
        emit_and(*groups[NU - 2])
        emit_v(NU - 3, groups[NU - 3][1])
        emit_and(*groups[NU - 1])
        emit_v(NU - 2, groups[NU - 2][1])
        emit_v(NU - 1, groups[NU - 1][1], final=True)
        emit_epilogue(1)

    nc.compile()
    return nc


def get_program():
    if "prog" not in _prog_cache:
        _prog_cache["prog"] = _build_program()
    return _prog_cache["prog"]


def make_in_maps(x, edge_index):
    fp8 = ml_dtypes.float8_e4m3
    x = np.asarray(x, dtype=np.float32)
    ei = np.asarray(edge_index)
    r = ei[0].astype(np.int64)
    c = ei[1].astype(np.int64)

    norm = np.sqrt((x * x).sum(axis=1, keepdims=True))
    nx16 = np.asarray((x / np.maximum(norm, 1e-12)) * 16.0, dtype=fp8)
    x8 = np.asarray(x, dtype=fp8)

    in_maps = []
    for k in range(NCORES):
        lo = k * R
        nxr = np.roll(nx16, -lo, axis=0)          # [N, D] fp8
        xar = np.roll(x8, -lo, axis=0)            # [N, D] fp8

        # xt[p, kt, c] = nxr[c, kt*128 + p]
        xt = np.ascontiguousarray(
            nxr.T.reshape(2, P, N).transpose(1, 0, 2)
        )
        # xa[p, g, kt, j] = xar[(2g+kt)*128 + p, j], ones at j=256
        xa = np.empty((P, NPAIR, 2, D + 1), dtype=fp8)
        xa[:, :, :, 0:D] = xar.reshape(NPAIR, 2, P, D).transpose(2, 0, 1, 3)
        xa[:, :, :, D] = fp8(-10.0)   # 1/pv[:,D] = -SCALE/S directly

        # keep-mask, rolled: mask[c_rolled, m_local] = 0 on edges
        sel = (r >= lo) & (r < lo + R)
        m_local = (r[sel] - lo).astype(np.int64)
        c_rolled = (c[sel] - lo) % N
        mask = np.full((N, R), 255, dtype=np.uint8)
        mask[c_rolled, m_local] = 0
        # mm[gg, s, kt, p, h, j] = mask[((gg*2+s)*2+kt)*128+p, h*512+j]
        mm = mask.reshape(NGRP, GRP, 2, P, 2, HALF)
        bias = np.where(mm == 0, B_EDGE, B_KEEP).astype(np.uint8)
        # mk[h, p, q*GPS+gg', s, (kt j)]: s=0 fp8 exp-bias (first pair),
        # s=1 AND-mask (second pair)
        sel2 = np.stack([bias[:, 0], mm[:, 1]], axis=1)   # [NGRP, 2, 2, P, 2, HALF]
        mk = np.ascontiguousarray(
            sel2.transpose(4, 3, 0, 1, 2, 5)              # [h, p, gg, s, kt, j]
            .reshape(2, P, NSLAB, GPS, 2, 2 * HALF)
        )
        xo = np.ascontiguousarray((1.0 + SCALE) * x[lo : lo + R])
        in_maps.append({"xt": xt, "xa": xa, "mk": mk, "xo": xo})
    return in_maps


def run(x, edge_index, trace=False):
    from concourse.bass_utils import run_bass_kernel_spmd

    nc = get_program()
    in_maps = make_in_maps(x, edge_index)
    br = run_bass_kernel_spmd(nc, in_maps, list(range(NCORES)), trace=trace)
    out = np.concatenate(
        [br.results[k]["out"] for k in range(NCORES)], axis=0
    ).astype(np.float32)
    return out, br


def kernel(x, edge_index):
    out, _ = run(x, edge_index, trace=False)
    return out


# revision 53
# speedup vs baseline: 1.0116x; 1.0116x over previous
"""ContraNorm kernel for 8 Trainium2 NeuronCores — fp8 DoubleRow pipeline.

Math (reference):
    norm_x = x / max(||x||_row, eps)
    sim    = (norm_x @ norm_x.T) / tau          # [N, N], tau = 1
    sim[edge_index[0], edge_index[1]] = -inf
    attn   = softmax(sim, axis=1)
    out    = 1.1 * x - 0.1 * (attn @ x)

Sharding: row-parallel.  Core k owns output rows [k*1024, (k+1)*1024).
Each core receives inputs row-rolled so its own rows sit at c-positions
0:1024 — the program is identical on every core (pure SPMD).

Since sim is a cosine similarity in [-1, 1], softmax needs no running
max.  fp8 (e4m3) everywhere on the matmul paths, DoubleRow perf mode:
  sim:  psum[c,m] = sum_{kt,dp} xt[dp,kt,c] * xt[dp,kt,m]   2 MM / pair
  V:    pv[m,:]  += sum_{kt,cp} et[cp,kt,m] * xa[cp,kt,:]   4 MM / pair
norm_x is pre-scaled by 16 on the host, so psum = 256*sim.  PE work
(1748ns per 2-pair group, ~56us/core warm) is the roofline; the whole
kernel runs as ONE flat 32-group software pipeline (both m-halves),
with exp+mask split so no other engine exceeds the PE:

  pair 0 of each group: TWO DVE scalar_tensor_tensor ops (one per
    single-bank psum tile) each do exp AND mask in one pass:
    u8 = psum*(8/ln2/256) + bias, bias fp8 {56 keep, -240 edge}.
    The u8 bits ARE fp8e4m3(exp(sim)) (Schraudolph: linear-in-mantissa
    exp2, error below the e4m3 quantization step after softmax);
    masked lanes go negative and saturate to 0x00 = +0.0.
  pair 1: TWO ACT exp ops (scale 1/256) -> fp8, then one u32
    bitwise-AND on DVE with a host-shipped {00,FF} keep-mask
    (4 bytes/cycle/lane; the old u8 multiply was 1B/cyc and measured
    70us of DVE — the original bottleneck).

Pipeline schedule (all measured-to-matter):
  - sim psum tiles are SINGLE-BANK [128,512], one per exp sub-op, so
    every psum write->consume->free round trip (~1.1us) sits far below
    the group period — the pipeline cannot fall into the slow
    psum-WAR limit cycle (which otherwise locks at 1964ns/group with
    one LDWEIGHTS-serialized 430ns matmul per group).
  - the AND runs two groups behind its ACT producer and after the
    current group's STTs in the DVE queue; V matmuls run three groups
    behind.  Nothing ever waits at an engine-queue head.
  - 8 warmup matmuls into the (later start=True-cleared) pv bank open
    the HAM clock gate (1.2 -> 2.4 GHz) before the main loop.
  - epilogue: ones column is -10 so DVE reciprocal yields -0.1/S;
    scale-apply alternates ACT Copy (per-partition scale AP) / DVE;
    the +1.1x combine runs on the otherwise-idle GpSimd (xo ships
    pre-scaled); outputs flush in four 256-row DMAs.

Per-core inputs (~13.5 MiB, ~38us DMA, overlapped with compute):
  xt    [128, 2, 8192] fp8     16*norm_x rolled, transposed
  xa    [128, 32, 2, 257] fp8  x rolled (V rhs layout) + (-10)-column
  mk    [2, 128, 4, 4, 2, 1024] u8  per half/partition/slab/group:
        [0] = fp8 exp-bias for pair 0, [1] = {00,FF} AND-mask for
        pair 1
  xo    [1024, 256] f32        1.1 * own rows for the epilogue
"""

import numpy as np
import ml_dtypes

N = 8192          # rows of x
D = 256           # features
P = 128           # SBUF partitions
NT = N // P       # 64 c-chunks
R = N // 8        # 1024 rows per core
HALF = 512        # m columns per pass
NPAIR = NT // 2   # 32 c-chunk pairs
SCALE = 0.1
NCORES = 8
GRP = 2           # pairs per group (1 ACT + 1 DVE)
NGRP = NPAIR // GRP
NSLAB = 4         # mask DMA slabs per half
GPS = NGRP // NSLAB
NWARM = 8         # PE warmup matmuls

SCH_A = 8.0 / np.log(2.0) / 256.0   # psum(=256*sim) -> fp8e4m3 bits slope
B_KEEP = 0x66                       # fp8 bits of +56  (= 8 * e4m3 bias 7)
B_EDGE = 0xF7                       # fp8 bits of -240 (saturates u8 to 0)

_prog_cache = {}


def _build_program():
    import concourse.bacc as bacc
    import concourse.tile as tile
    from concourse import mybir
    from contextlib import ExitStack

    f32 = mybir.dt.float32
    fp8 = mybir.dt.float8e4
    u32 = mybir.dt.uint32
    u8 = mybir.dt.uint8
    DR = mybir.MatmulPerfMode.DoubleRow
    Exp = mybir.ActivationFunctionType.Exp
    Copy = mybir.ActivationFunctionType.Copy
    AND = mybir.AluOpType.bitwise_and
    MUL = mybir.AluOpType.mult
    ADD = mybir.AluOpType.add

    nc = bacc.Bacc("TRN2", target_bir_lowering=False, debug=False)

    xt_h = nc.dram_tensor("xt", [P, 2, N], fp8, kind="ExternalInput")
    xa_h = nc.dram_tensor("xa", [P, NPAIR, 2, D + 1], fp8, kind="ExternalInput")
    mk_h = nc.dram_tensor(
        "mk", [2, P, NSLAB, GPS, 2, 2 * HALF], u8, kind="ExternalInput"
    )
    xo_h = nc.dram_tensor("xo", [R, D], f32, kind="ExternalInput")
    out_h = nc.dram_tensor("out", [R, D], f32, kind="ExternalOutput")

    xo_d = xo_h.ap().rearrange("(j p) d -> p j d", p=P)    # [128, 8, 256]
    out_d = out_h.ap()

    with ExitStack() as ctx:
        tc = ctx.enter_context(tile.TileContext(nc))

        consts = ctx.enter_context(tc.tile_pool(name="consts", bufs=1))
        work = ctx.enter_context(tc.tile_pool(name="work", bufs=4))
        # single-bank sim psum tiles: each STT/ACT sub-op frees its bank
        # ~1.1us after the sim matmul wrote it — every psum round-trip sits
        # far below the 1.75us group period, so the pipeline cannot fall
        # into the slow (psum-WAR -> cold-matmul) limit cycle
        ps_a = ctx.enter_context(tc.tile_pool(name="ps_a", bufs=2, space="PSUM"))
        ps_b = ctx.enter_context(tc.tile_pool(name="ps_b", bufs=2, space="PSUM"))
        ps_v = ctx.enter_context(tc.tile_pool(name="ps_v", bufs=1, space="PSUM"))

        xt = consts.tile([P, 2, N], fp8)
        xa = consts.tile([P, NPAIR, 2, D + 1], fp8)
        xo = consts.tile([P, R // P, D], f32)
        mk = [
            [
                consts.tile([P, GPS, 2, 2 * HALF], u8, name=f"mk{h}{q}")
                for q in range(NSLAB)
            ]
            for h in range(2)
        ]

        # DMA order paces arrival against consumption (~38us total).
        def dx(c0, c1):
            nc.sync.dma_start(out=xt[:, :, c0:c1], in_=xt_h.ap()[:, :, c0:c1])

        XAC = NPAIR // 4
        dx(0, 8 * P)
        # first mask slab ships in two halves so group 0's Schraudolph STT
        # fires ~1.5us earlier (the ramp head is DMA-latency-bound)
        nc.sync.dma_start(out=mk[0][0][:, 0:2], in_=mk_h.ap()[0, :, 0, 0:2])
        dx(8 * P, 16 * P)
        nc.sync.dma_start(out=mk[0][0][:, 2:4], in_=mk_h.ap()[0, :, 0, 2:4])
        nc.sync.dma_start(out=xa[:, 0:XAC], in_=xa_h.ap()[:, 0:XAC])
        nc.sync.dma_start(out=mk[0][1], in_=mk_h.ap()[0, :, 1])
        dx(16 * P, 32 * P)
        nc.sync.dma_start(out=xa[:, XAC:2 * XAC], in_=xa_h.ap()[:, XAC:2 * XAC])
        nc.sync.dma_start(out=mk[0][2], in_=mk_h.ap()[0, :, 2])
        dx(32 * P, 48 * P)
        nc.sync.dma_start(out=xa[:, 2 * XAC:3 * XAC], in_=xa_h.ap()[:, 2 * XAC:3 * XAC])
        nc.sync.dma_start(out=mk[0][3], in_=mk_h.ap()[0, :, 3])
        dx(48 * P, N)
        # the first h1 mask slab's deadline (group 16, ~41us) is ~6us
        # tighter than xa3/xo's — ship it ahead of them (measured +5us
        # stall at the half boundary otherwise)
        nc.sync.dma_start(out=mk[1][0], in_=mk_h.ap()[1, :, 0])
        nc.sync.dma_start(out=xa[:, 3 * XAC:NPAIR], in_=xa_h.ap()[:, 3 * XAC:NPAIR])
        nc.sync.dma_start(out=xo, in_=xo_d)
        for q in range(1, NSLAB):
            nc.sync.dma_start(out=mk[1][q], in_=mk_h.ap()[1, :, q])

        pvh = {}

        def emit_and(pg, et4m_p, et2_p):
            # mask for the ACT pair, pipelined two groups back and emitted
            # after the current group's STTs — its et2 input is long done,
            # so it never couples the ACT exp latency into the DVE queue
            # ahead of a Schraudolph STT
            if et2_p is None:
                return
            mks = mk[pg // NGRP][(pg % NGRP) // GPS]
            gl = pg % GPS
            nc.vector.tensor_tensor(
                out=et4m_p[:, 1].rearrange("p a b -> p (a b)").bitcast(u32),
                in0=et2_p.rearrange("p a b -> p (a b)").bitcast(u32),
                in1=mks[:, gl, 1, :].bitcast(u32),
                op=AND,
            )

        def emit_v(pg, et, final=False):
            # V matmuls for group pg (software-pipelined three groups back
            # so the PE never queues behind the exp->mask chain).
            # mi-major on the last group: pv[0] stops earlier, so the
            # epilogue chain starts while the tail matmuls run.
            pv = pvh[pg // NGRP]
            gg = pg % NGRP
            order = (
                [(gi, mi) for mi in range(4) for gi in range(GRP)]
                if final
                else [(gi, mi) for gi in range(GRP) for mi in range(4)]
            )
            for gi, mi in order:
                g = gg * GRP + gi
                nc.tensor.matmul(
                    pv[mi],
                    et[:, gi, :, mi * P : (mi + 1) * P],
                    xa[:, g],
                    start=(g == 0),
                    stop=(g == NPAIR - 1),
                    perf_mode=DR,
                )

        def emit_epilogue(h):
            # out = 1.1*x - 0.1 * pv/S; ones column is -10, so
            # pv[:,D] = -10*S and reciprocal gives -0.1/S directly
            pv = pvh[h]
            resh = work.tile([P, 4, D], f32, tag="resh", bufs=2)
            for mi in range(4):
                jj = h * 4 + mi
                sinv = work.tile([P, 1], f32, tag="sinv")
                nc.vector.reciprocal(sinv, pv[mi][:, D : D + 1])
                # split the pv evacuation 2/2 between ACT and DVE (gpsimd
                # cannot read PSUM): a 4-Copy burst on ACT alone delays the
                # next half's exps behind it in the FIFO
                if mi % 2 == 0:
                    nc.scalar.activation(
                        resh[:, mi],
                        pv[mi][:, 0:D],
                        Copy,
                        scale=sinv,
                    )
                else:
                    nc.vector.tensor_scalar(
                        out=resh[:, mi],
                        in0=pv[mi][:, 0:D],
                        scalar1=sinv,
                        scalar2=1.0,
                        op0=MUL,
                        op1=MUL,
                    )
                # float combine on the idle GpSimd engine: keeps the DVE
                # budget (2 STT + AND ~ 1742ns/period) from overflowing at
                # the epilogue.  xo ships pre-scaled by 1.1 so this is a
                # plain add (gpsimd has no scalar_tensor_tensor opcode).
                nc.gpsimd.tensor_tensor(
                    out=resh[:, mi],
                    in0=xo[:, jj],
                    in1=resh[:, mi],
                    op=ADD,
                )
                if mi % 2 == 1:
                    # flush in 256-row chunks so the last output DMA is
                    # small (tail) and the first overlaps the epilogue
                    j0 = h * 4 + mi - 1
                    nc.sync.dma_start(
                        out=out_d[j0 * P : (j0 + 2) * P, :].rearrange(
                            "(j p) d -> p j d", p=P
                        ),
                        in_=resh[:, mi - 1 : mi + 1],
                    )

        # single flat 32-group software pipeline across both halves — no
        # drain/refill dip at the half boundary
        groups = []
        NU = 2 * NGRP
        for u in range(NU):
            h, gg = divmod(u, NGRP)
            m0 = h * HALF
            if gg == 0:
                pvh[h] = [
                    ps_v.tile([P, D + 1], f32, tag=f"pv{mi}", name=f"pv{h}{mi}")
                    for mi in range(4)
                ]
            if u == 0:
                # PE warmup: junk matmuls into pv0 (cleared by the real
                # start=True) so the HAM clock gate opens before the
                # main loop needs full throughput.
                for _ in range(NWARM):
                    nc.tensor.matmul(
                        pvh[0][0][:, 0:D],
                        xt[:, 0, 0:P],
                        xt[:, 0, 0:2 * P],
                        start=True,
                        stop=True,
                    )
            et4m = work.tile([P, GRP, 2, HALF], fp8, tag="et4m", bufs=5)
            mks = mk[h][gg // GPS]
            gl = gg % GPS
            et2 = None
            for gi in range(GRP):
                g = gg * GRP + gi
                stt_pair = gi == 0
                if not stt_pair:
                    et2 = work.tile([P, 2, HALF], fp8, tag="et2", bufs=5)
                for kt in range(2):
                    t = 2 * g + kt
                    pool = ps_a if gi == 0 else ps_b
                    pss = pool.tile([P, HALF], f32, tag=f"ps{gi}")
                    nc.tensor.matmul(
                        pss,
                        xt[:, :, t * P : (t + 1) * P],
                        xt[:, :, m0 : m0 + HALF],
                        start=True,
                        stop=True,
                        perf_mode=DR,
                    )
                    if stt_pair:
                        # one DVE op = Schraudolph exp + mask (first pair
                        # always; both pairs in the very last group so the
                        # tail never waits on ACT exp + AND)
                        nc.vector.scalar_tensor_tensor(
                            out=et4m[:, gi, kt, :].bitcast(u8),
                            in0=pss,
                            scalar=float(SCH_A),
                            in1=mks[:, gl, gi, kt * HALF : (kt + 1) * HALF].bitcast(fp8),
                            op0=MUL,
                            op1=ADD,
                        )
                    else:
                        # second pair: ACT exp -> fp8, masked by emit_and
                        nc.scalar.activation(
                            et2[:, kt, :],
                            pss,
                            Exp,
                            scale=1.0 / 256.0,
                        )
            groups.append((u, et4m, et2))
            if u >= 2:
                emit_and(*groups[u - 2])
            if u >= 3:
                emit_v(u - 3, groups[u - 3][1])
                if (u - 3) % NGRP == NGRP - 1:
                    emit_epilogue((u - 3) // NGRP)
        # tail: ANDs as early as their ACT inputs allow, so the trailing
        # V groups and the epilogue recips behind them on DVE stream
        # without queue-head waits
        emit_and(*groups[NU - 2])
        emit_v(NU - 3, groups[NU - 3][1])
        emit_and(*groups[NU - 1])
        emit_v(NU - 2, groups[NU - 2][1])
        emit_v(NU - 1, groups[NU - 1][1], final=True)
        emit_epilogue(1)

    nc.compile()
    return nc


def get_program():
    if "prog" not in _prog_cache:
        _prog_cache["prog"] = _build_program()
    return _prog_cache["prog"]


def make_in_maps(x, edge_index):
    fp8 = ml_dtypes.float8_e4m3
    x = np.asarray(x, dtype=np.float32)
    ei = np.asarray(edge_index)
    r = ei[0].astype(np.int64)
    c = ei[1].astype(np.int64)

    norm = np.sqrt((x * x).sum(axis=1, keepdims=True))
    nx16 = np.asarray((x / np.maximum(norm, 1e-12)) * 16.0, dtype=fp8)
    x8 = np.asarray(x, dtype=fp8)

    in_maps = []
    for k in range(NCORES):
        lo = k * R
        nxr = np.roll(nx16, -lo, axis=0)          # [N, D] fp8
        xar = np.roll(x8, -lo, axis=0)            # [N, D] fp8

        # xt[p, kt, c] = nxr[c, kt*128 + p]
        xt = np.ascontiguousarray(
            nxr.T.reshape(2, P, N).transpose(1, 0, 2)
        )
        # xa[p, g, kt, j] = xar[(2g+kt)*128 + p, j], ones at j=256
        xa = np.empty((P, NPAIR, 2, D + 1), dtype=fp8)
        xa[:, :, :, 0:D] = xar.reshape(NPAIR, 2, P, D).transpose(2, 0, 1, 3)
        xa[:, :, :, D] = fp8(-10.0)   # 1/pv[:,D] = -SCALE/S directly

        # keep-mask, rolled: mask[c_rolled, m_local] = 0 on edges
        sel = (r >= lo) & (r < lo + R)
        m_local = (r[sel] - lo).astype(np.int64)
        c_rolled = (c[sel] - lo) % N
        mask = np.full((N, R), 255, dtype=np.uint8)
        mask[c_rolled, m_local] = 0
        # mm[gg, s, kt, p, h, j] = mask[((gg*2+s)*2+kt)*128+p, h*512+j]
        mm = mask.reshape(NGRP, GRP, 2, P, 2, HALF)
        bias = np.where(mm == 0, B_EDGE, B_KEEP).astype(np.uint8)
        # mk[h, p, q*GPS+gg', s, (kt j)]: s=0 fp8 exp-bias (first pair),
        # s=1 AND-mask (second pair)
        sel2 = np.stack([bias[:, 0], mm[:, 1]], axis=1)   # [NGRP, 2, 2, P, 2, HALF]
        mk = np.ascontiguousarray(
            sel2.transpose(4, 3, 0, 1, 2, 5)              # [h, p, gg, s, kt, j]
            .reshape(2, P, NSLAB, GPS, 2, 2 * HALF)
        )
        xo = np.ascontiguousarray((1.0 + SCALE) * x[lo : lo + R])
        in_maps.append({"xt": xt, "xa": xa, "mk": mk, "xo": xo})
    return in_maps


def run(x, edge_index, trace=False):
    from concourse.bass_utils import run_bass_kernel_spmd

    nc = get_program()
    in_maps = make_in_maps(x, edge_index)
    br = run_bass_kernel_spmd(nc, in_maps, list(range(NCORES)), trace=trace)
    out = np.concatenate(
        [br.results[k]["out"] for k in range(NCORES)], axis=0
    ).astype(np.float32)
    return out, br


def kernel(x, edge_index):
    out, _ = run(x, edge_index, trace=False)
    return out
